# revision 4
# baseline (speedup 1.0000x reference)
"""Trainium2 Bass kernel for nn_Attention_51548197487430.

Multi-head attention (B=2, S=2048, D=1024, H=16, HD=64), fp32, sharded
2 heads per core across 8 NeuronCores (head/tensor parallel per the
sharding hint: w_qkv output dim and w_out input dim split per-head; the
all-reduce after the output projection is realized as the host-side
unshard step, which sums the 8 partial outputs).

Per-core device kernel (SPMD, identical program; per-core weight slices):
  inputs (host pre-laid-out):
    xT     (1024, 4096)  x^T, both batches side by side   [same on all cores]
    wqkvT  (1024, 384)   [Wq_h0|Wq_h1|Wk_h0|Wk_h1|Wv_h0|Wv_h1]^T for this core
    woutT  (128, 1024)   w_out[:, 128c:128c+128]^T
  output:
    out    (4096, 1024)  partial x_out (pre-bias); host sums over cores

  1. QKV^T projection: QT/KT/VT (128, 4096), fp32r matmuls (full speed).
  2. V^T -> V via PE transpose; build V_aug (per head): h0=[V|ones],
     h1=[ones|V] so the ones-columns compute the softmax denominator in
     the same matmul that computes attn@V, landing on the partition
     half opposite to o^T.
  3. Per (batch, q-512-chunk): loop over 16 k-chunks: scores^T by
     row-packed matmul pairs (two heads in row groups 0-1/2-3), one
     fused exp over both heads' psum banks (scale=0.125 folded in; max
     subtraction skipped - scores are O(+-6), exp is safe in fp32),
     attn@V accumulation.  Softmax normalization via PE broadcast of the
     denominator row + DVE reciprocal + multiply.
  4. Output projection (contraction over this core's 128 o-dims).
"""

import numpy as np

B, S, D, H, HD = 2, 2048, 1024, 16, 64
N_CORES = 8
SCALE = HD ** (-0.5)
BS = B * S               # 4096
SC = 512                 # qkv-phase s-chunk (8 chunks)
QC = 512                 # attention q-chunk
NKC = S // 128           # 16 k-chunks per batch
DC = D // 128            # 8 contraction chunks

_cache = {}


def _build():
    import concourse.bass as bass
    import concourse.mybir as mybir
    import concourse.tile as tile
    from concourse import bacc

    F32 = mybir.dt.float32
    F32R = mybir.dt.float32r
    AF = mybir.ActivationFunctionType

    nc = bacc.Bacc("TRN2", target_bir_lowering=False, debug=False,
                   num_devices=N_CORES)
    xT_d = nc.dram_tensor("xT", (D, BS), F32R, kind="ExternalInput").ap()
    wqkvT_d = nc.dram_tensor("wqkvT", (D, 384), F32R, kind="ExternalInput").ap()
    woutT_d = nc.dram_tensor("woutT", (128, D), F32R, kind="ExternalInput").ap()
    out_d = nc.dram_tensor("out", (BS, D), F32, kind="ExternalOutput").ap()

    with tile.TileContext(nc) as tc:
        with tc.tile_pool(name="persist", bufs=1) as persist, \
             tc.tile_pool(name="xin", bufs=2) as xin, \
             tc.tile_pool(name="epool", bufs=3) as epool, \
             tc.tile_pool(name="work", bufs=2) as work, \
             tc.tile_pool(name="ps_sc", bufs=2, space="PSUM") as ps_sc, \
             tc.tile_pool(name="pacc", bufs=4, space="PSUM") as pacc:

            # ---- persistent tiles ----
            wqkvT = persist.tile([128, DC, 384], F32R, tag="wqkvT")
            nc.sync.dma_start(wqkvT[:], wqkvT_d.rearrange("(po pi) e -> pi po e", pi=128))
            woutT = persist.tile([128, D], F32R, tag="woutT")
            nc.sync.dma_start(woutT[:], woutT_d)

            ident = persist.tile([128, 128], F32, tag="ident")
            from concourse.masks import make_identity
            make_identity(nc, ident[:])

            QT = persist.tile([128, BS], F32R, tag="QT")
            KT = persist.tile([128, BS], F32R, tag="KT")
            VT = persist.tile([128, BS], F32, tag="VT")
            # V_aug[b][h]: (128, NKC, 128); h0 = [V | ones], h1 = [ones | V]
            vaug = [[persist.tile([128, NKC, 128], F32R, tag=f"vaug{b}{h}",
                                  name=f"vaug{b}{h}")
                     for h in range(2)] for b in range(B)]
            # constants: memset supports fp32 only; DVE copy rounds to f32r
            const_f32 = persist.tile([128, NKC * 64], F32, tag="const_f32")
            nc.gpsimd.memset(const_f32[:], 1.0)
            inv64 = persist.tile([128, 128], F32R, tag="inv64")
            inv64_f32 = persist.tile([128, 128], F32, tag="inv64_f32")
            nc.gpsimd.memset(inv64_f32[:], 1.0 / 64.0)
            nc.vector.tensor_copy(inv64[:], inv64_f32[:])
            ones_3d = const_f32[:].rearrange("p (a b) -> p a b", b=64)
            for b in range(B):
                nc.vector.tensor_copy(vaug[b][0][:, :, 64:128], ones_3d)
                nc.vector.tensor_copy(vaug[b][1][:, :, 0:64], ones_3d)

            def emit_qkv(s):
                """Project x^T cols [s*512, (s+1)*512) -> QT/KT/VT."""
                xt = xin.tile([128, DC, SC], F32R, tag="xt")
                nc.sync.dma_start(
                    xt[:], xT_d[:, s * SC:(s + 1) * SC]
                    .rearrange("(po pi) s -> pi po s", pi=128))
                for e, dst in ((0, QT), (1, KT), (2, VT)):
                    ps = pacc.tile([128, SC], F32, tag="pacc")
                    for d in range(DC):
                        nc.tensor.matmul(
                            ps[:], lhsT=wqkvT[:, d, 128 * e:128 * (e + 1)],
                            rhs=xt[:, d, :], start=(d == 0), stop=(d == DC - 1))
                    nc.vector.tensor_copy(dst[:, s * SC:(s + 1) * SC], ps[:])

            def emit_vtrans(j):
                """Transpose VT cols [j*128,(j+1)*128) into V_aug tiles."""
                b, k = divmod(j, NKC)
                ps = pacc.tile([128, SC], F32, tag="pacc")
                pt = ps[:, 0:128]
                nc.tensor.transpose(pt, VT[:, j * 128:(j + 1) * 128], ident[:])
                nc.vector.tensor_copy(vaug[b][0][:, k, 0:64], pt[:, 0:64])
                nc.vector.tensor_copy(vaug[b][1][:, k, 64:128], pt[:, 64:128])

            def emit_attn(b, q):
                """One (batch, q-512-chunk): softmax(QK^T*scale)V + out proj."""
                q0 = b * S + q * QC          # column base in QT (global s)
                psA = pacc.tile([128, SC], F32, tag="pacc")  # h0: oT | denom
                psB = pacc.tile([128, SC], F32, tag="pacc")  # h1: denom | oT
                for k in range(NKC):
                    kcol = b * S + k * 128
                    pss = ps_sc.tile([128, 2 * QC], F32, tag="scores")
                    # scores^T, two heads packed in row groups (K=64 each)
                    nc.tensor.matmul(
                        pss[:, 0:QC], lhsT=KT[0:64, kcol:kcol + 128],
                        rhs=QT[0:64, q0:q0 + QC], start=True, stop=True)
                    nc.tensor.matmul(
                        pss[:, QC:2 * QC], lhsT=KT[64:128, kcol:kcol + 128],
                        rhs=QT[64:128, q0:q0 + QC], start=True, stop=True)
                    eb = epool.tile([128, 2 * QC], F32R, tag="eb")
                    nc.scalar.activation(eb[:], pss[:], AF.Exp, scale=float(SCALE))
                    nc.tensor.matmul(psA[:], lhsT=vaug[b][0][:, k, :],
                                     rhs=eb[:, 0:QC],
                                     start=(k == 0), stop=(k == NKC - 1))
                    nc.tensor.matmul(psB[:], lhsT=vaug[b][1][:, k, :],
                                     rhs=eb[:, QC:2 * QC],
                                     start=(k == 0), stop=(k == NKC - 1))
                # denominators -> SBUF (same partitions as they sit in PSUM)
                den = work.tile([128, QC], F32R, tag="den")
                nc.vector.tensor_copy(den[64:128, :], psA[64:128, :])
                nc.vector.tensor_copy(den[0:64, :], psB[0:64, :])
                # broadcast denom to all partitions: ones(64,128)/64 ^T @ den
                invd = work.tile([128, QC], F32, tag="invd")
                pbc = pacc.tile([128, SC], F32, tag="pacc")
                nc.tensor.matmul(pbc[:], lhsT=inv64[64:128, :],
                                 rhs=den[64:128, :], start=True, stop=True)
                nc.vector.reciprocal(invd[0:64, :], pbc[0:64, :])
                pbc2 = pacc.tile([128, SC], F32, tag="pacc")
                nc.tensor.matmul(pbc2[:], lhsT=inv64[0:64, :],
                                 rhs=den[0:64, :], start=True, stop=True)
                nc.vector.reciprocal(invd[64:128, :], pbc2[64:128, :])
                # normalized o^T (128 = both heads' dims, QC)
                ot = work.tile([128, QC], F32R, tag="ot")
                nc.vector.tensor_mul(out=ot[0:64, :], in0=psA[0:64, :],
                                     in1=invd[0:64, :])
                nc.vector.tensor_mul(out=ot[64:128, :], in0=psB[64:128, :],
                                     in1=invd[64:128, :])
                # output projection: out[s, :] = ot[:, s].T @ woutT
                for j in range(QC // 128):
                    osb = work.tile([128, D], F32, tag="osb")
                    for e in range(D // SC):
                        po = pacc.tile([128, SC], F32, tag="pacc")
                        nc.tensor.matmul(
                            po[:], lhsT=ot[:, j * 128:(j + 1) * 128],
                            rhs=woutT[:, e * SC:(e + 1) * SC],
                            start=True, stop=True)
                        nc.vector.tensor_copy(osb[:, e * SC:(e + 1) * SC], po[:])
                    row = q0 + j * 128
                    nc.sync.dma_start(out_d[row:row + 128, :], osb[:])

            # ---- emission order: overlap qkv(b1) under attn(b0) ----
            for s in range(4):
                emit_qkv(s)
            for j in range(NKC):
                emit_vtrans(j)
            for q in range(4):
                emit_attn(0, q)
                if q < 4:
                    s = 4 + q
                    emit_qkv(s)
                    for j in range(NKC + 4 * q, NKC + 4 * (q + 1)):
                        emit_vtrans(j)
            for q in range(4):
                emit_attn(1, q)

    nc.compile()
    return nc


def _get_nc():
    if "nc" not in _cache:
        _cache["nc"] = _build()
    return _cache["nc"]


def kernel(x, w_qkv, w_out, b_out):
    from concourse.bass_utils import run_bass_kernel_spmd

    nc = _get_nc()
    x = np.ascontiguousarray(np.asarray(x, dtype=np.float32))
    w_qkv = np.asarray(w_qkv, dtype=np.float32)
    w_out = np.asarray(w_out, dtype=np.float32)
    b_out = np.asarray(b_out, dtype=np.float32)

    xT = np.ascontiguousarray(x.reshape(BS, D).T)          # (D, BS)
    in_maps = []
    for c in range(N_CORES):
        # reference splits qkv as (v, q, k): v rows [0,D), q [D,2D), k [2D,3D)
        wq = w_qkv[D + 128 * c: D + 128 * (c + 1)]
        wk = w_qkv[2 * D + 128 * c: 2 * D + 128 * (c + 1)]
        wv = w_qkv[128 * c: 128 * (c + 1)]
        wqkvT = np.ascontiguousarray(np.concatenate([wq, wk, wv], axis=0).T)
        woutT = np.ascontiguousarray(w_out[:, 128 * c:128 * (c + 1)].T)
        in_maps.append({"xT": xT, "wqkvT": wqkvT, "woutT": woutT})

    res = run_bass_kernel_spmd(nc, in_maps, core_ids=list(range(N_CORES)))
    acc = res.results[0]["out"]
    for c in range(1, N_CORES):
        acc = acc + res.results[c]["out"]
    acc = acc + b_out[None, :]
    return acc.reshape(B, S, D)


# revision 25
# speedup vs baseline: 1.5197x; 1.5197x over previous
"""Trainium2 Bass kernel for nn_Attention_51548197487430.

Multi-head attention (B=2, S=2048, D=1024, H=16, HD=64), fp32 reference,
sharded 2 heads per core across 8 NeuronCores (head/tensor parallel per
the sharding hint: w_qkv output dim and w_out input dim split per-head;
the all-reduce after the output projection is realized as the host-side
unshard step, which sums the 8 partial outputs).

Per-core device kernel (SPMD, identical program; per-core weight slices):
  inputs (host pre-laid-out, cast to bf16):
    xT     (1024, 4096)  x^T, both batches side by side   [same on all cores]
    wqkvT  (1024, 384)   [Wq_h0|Wq_h1|Wk_h0|Wk_h1|Wv_h0|Wv_h1]^T for this core
    woutT  (128, 1024)   w_out[:, 128c:128c+128]^T
  output:
    out    (4096, 1024)  fp32 partial x_out (pre-bias); host sums over cores

  1. QKV^T projection -> QT/KT/VT (128, 4096); matmuls bf16 (fp32 PSUM).
  2. V^T -> V via PE transpose; build V_aug per head: h0=[V|ones],
     h1=[ones|V] - the ones-columns compute the softmax denominator in
     the same matmul as attn@V, on the partition half opposite to o^T.
  3. Per (batch, q-512-chunk), loop 16 k-chunks: scores^T by row-packed
     matmul pairs (two heads in PE row groups 0-1/2-3 concurrently), one
     fused exp over both heads' psum banks (scale folded in; max
     subtraction skipped - scores are O(+-6) so fp32 exp is safe),
     attn@V accumulation.  Normalization: PE broadcast of the denominator
     rows + DVE reciprocal + multiply, off the critical path.
  4. Output projection (contraction over this core's 128 o-dims).
"""

import numpy as np

B, S, D, H, HD = 2, 2048, 1024, 16, 64
N_CORES = 8
SCALE = HD ** (-0.5)
BS = B * S               # 4096
SC = 512                 # qkv-phase s-chunk (8 chunks)
QC = 512                 # attention q-chunk
NKC = S // 128           # 16 k-chunks per batch
DC = D // 128            # 8 contraction chunks

_cache = {}


def _build():
    import concourse.bass as bass
    import concourse.mybir as mybir
    import concourse.tile as tile
    from concourse import bacc

    F32 = mybir.dt.float32
    F32R = mybir.dt.float32r
    BF16 = mybir.dt.bfloat16
    F16 = mybir.dt.float16
    AF = mybir.ActivationFunctionType

    nc = bacc.Bacc("TRN2", target_bir_lowering=False, debug=False,
                   num_devices=N_CORES)
    xT_d = nc.dram_tensor("xT", (D, BS), BF16, kind="ExternalInput").ap()
    wqkvT_d = nc.dram_tensor("wqkvT", (D, 384), BF16, kind="ExternalInput").ap()
    woutT_d = nc.dram_tensor("woutT", (128, D), BF16, kind="ExternalInput").ap()
    # fp16 partials: |values| << 1, so fp16's 10-bit mantissa beats bf16
    # and halves the output DMA; host upcasts and sums in fp32.
    out_d = nc.dram_tensor("out", (BS, D), F16, kind="ExternalOutput").ap()

    with tile.TileContext(nc) as tc:
        with tc.tile_pool(name="persist", bufs=1) as persist, \
             tc.tile_pool(name="xin", bufs=2) as xin, \
             tc.tile_pool(name="epool", bufs=3) as epool, \
             tc.tile_pool(name="work", bufs=2) as work, \
             tc.tile_pool(name="ps_sc", bufs=2, space="PSUM") as ps_sc, \
             tc.tile_pool(name="pacc", bufs=4, space="PSUM") as pacc:

            # ---- persistent tiles ----
            wqkvT = persist.tile([128, DC, 384], BF16, tag="wqkvT")
            nc.sync.dma_start(wqkvT[:], wqkvT_d.rearrange("(po pi) e -> pi po e", pi=128))
            woutT = persist.tile([128, D], BF16, tag="woutT")
            nc.sync.dma_start(woutT[:], woutT_d)

            ident = persist.tile([128, 128], F32, tag="ident")
            from concourse.masks import make_identity
            make_identity(nc, ident[:])

            QT = persist.tile([128, BS], BF16, tag="QT")
            KT = persist.tile([128, BS], BF16, tag="KT")
            VT = persist.tile([128, BS], F32, tag="VT")
            # V_aug[b][h]: (128, NKC, 128); h0 = [V | ones], h1 = [ones | V]
            vaug = [[persist.tile([128, NKC, 128], BF16, tag=f"vaug{b}{h}",
                                  name=f"vaug{b}{h}")
                     for h in range(2)] for b in range(B)]
            # constants: memset supports fp32 only; DVE copy rounds/casts
            const_f32 = persist.tile([128, NKC * 64], F32, tag="const_f32")
            nc.gpsimd.memset(const_f32[:], 1.0)
            # inv2: anti-block-diagonal 1/64 weights; one matmul pair
            # accumulates both heads' denominator broadcasts into ONE psum
            # tile (h0's denom -> partitions 0-63, h1's -> 64-127).
            inv2 = persist.tile([128, 128], F32R, tag="inv2")
            inv2_f32 = persist.tile([128, 128], F32, tag="inv2_f32")
            nc.gpsimd.memset(inv2_f32[:], 0.0)
            nc.gpsimd.memset(inv2_f32[64:128, 0:64], 1.0 / 64.0)
            nc.gpsimd.memset(inv2_f32[0:64, 64:128], 1.0 / 64.0)
            nc.vector.tensor_copy(inv2[:], inv2_f32[:])
            ones_3d = const_f32[:].rearrange("p (a b) -> p a b", b=64)
            for b in range(B):
                nc.vector.tensor_copy(vaug[b][0][:, :, 64:128], ones_3d)
                nc.vector.tensor_copy(vaug[b][1][:, :, 0:64], ones_3d)

            xts = {}

            def emit_xt_dma(s):
                xt = xin.tile([128, DC, SC], BF16, tag="xt", name="xt")
                nc.sync.dma_start(
                    xt[:], xT_d[:, s * SC:(s + 1) * SC]
                    .rearrange("(po pi) s -> pi po s", pi=128))
                xts[s] = xt

            def emit_qkv_part(s, e):
                """One e-chunk (Q, K or V) of the projection for s-chunk s."""
                dst = (QT, KT, VT)[e]
                ps = pacc.tile([128, SC], F32, tag="pacc", name="qkv_ps")
                for d in range(DC):
                    nc.tensor.matmul(
                        ps[:], lhsT=wqkvT[:, d, 128 * e:128 * (e + 1)],
                        rhs=xts[s][:, d, :], start=(d == 0), stop=(d == DC - 1))
                nc.vector.tensor_copy(dst[:, s * SC:(s + 1) * SC], ps[:])

            def emit_qkv(s):
                """Project x^T cols [s*512, (s+1)*512) -> QT/KT/VT."""
                emit_xt_dma(s)
                for e in range(3):
                    emit_qkv_part(s, e)

            def emit_vtrans(j):
                """Transpose VT cols [j*128,(j+1)*128) into V_aug tiles."""
                b, k = divmod(j, NKC)
                ps = pacc.tile([128, SC], F32, tag="pacc")
                pt = ps[:, 0:128]
                nc.tensor.transpose(pt, VT[:, j * 128:(j + 1) * 128], ident[:])
                nc.vector.tensor_copy(vaug[b][0][:, k, 0:64], pt[:, 0:64])
                nc.vector.tensor_copy(vaug[b][1][:, k, 64:128], pt[:, 64:128])

            def emit_finish_stage(st, stage):
                """Software-pipelined tail of a previous attention iteration,
                interleaved into the current k-loop so the PE's static
                instruction order never blocks on the DVE normalize chain."""
                if st is None:
                    return
                if stage == 0:
                    # both heads' denominator broadcasts accumulate into ONE
                    # psum tile (anti-block-diagonal inv2) -> one reciprocal
                    st["invd"] = work.tile([128, QC], F32, tag="invd", name="invd")
                    st["pbc"] = pacc.tile([128, SC], F32, tag="pacc", name="pbc")
                    nc.tensor.matmul(st["pbc"][:], lhsT=inv2[64:128, :],
                                     rhs=st["odA"][64:128, :],
                                     start=True, stop=False)
                    nc.tensor.matmul(st["pbc"][:], lhsT=inv2[0:64, :],
                                     rhs=st["odB"][0:64, :],
                                     start=False, stop=True)
                    nc.vector.reciprocal(st["invd"][:], st["pbc"][:])
                elif stage == 1:
                    st["ot"] = work.tile([128, QC], BF16, tag="ot", name="ot")
                    nc.vector.tensor_mul(out=st["ot"][0:64, :],
                                         in0=st["odA"][0:64, :].bitcast(F32),
                                         in1=st["invd"][0:64, :])
                    nc.vector.tensor_mul(out=st["ot"][64:128, :],
                                         in0=st["odB"][64:128, :].bitcast(F32),
                                         in1=st["invd"][64:128, :])
                else:
                    # stages 2..5: output projection, one 128-row chunk each
                    j = stage - 2
                    osb = work.tile([128, D], F16, tag="osb")
                    for e in range(D // SC):
                        po = pacc.tile([128, SC], F32, tag="pacc")
                        nc.tensor.matmul(
                            po[:], lhsT=st["ot"][:, j * 128:(j + 1) * 128],
                            rhs=woutT[:, e * SC:(e + 1) * SC],
                            start=True, stop=True)
                        nc.vector.tensor_copy(osb[:, e * SC:(e + 1) * SC], po[:])
                    row = st["q0"] + j * 128
                    nc.sync.dma_start(out_d[row:row + 128, :], osb[:])

            # k-index -> pipelined finish stage of the previous iteration
            # (stages 4-5 = outproj j2/j3 run at the iteration boundary
            # as PE filler while the accumulators drain)
            FIN_AT = {4: 0, 6: 1, 8: 2, 10: 3}

            def emit_attn(b, q, prev, filler=None):
                """One (batch, q-512-chunk): softmax(QK^T*scale)V.  `filler`
                maps k -> list of closures (qkv/vtrans work spread through the
                k-loop to even out PE duty).  Returns state for the pipelined
                finish (normalize + out projection)."""
                filler = filler or {}
                q0 = b * S + q * QC          # column base in QT (global s)
                psA = pacc.tile([128, SC], F32, tag="pacc")  # h0: oT | denom
                psB = pacc.tile([128, SC], F32, tag="pacc")  # h1: denom | oT
                for k in range(NKC):
                    kcol = b * S + k * 128
                    pss = ps_sc.tile([128, 2 * QC], F32, tag="scores")
                    # scores^T, two heads packed in row groups (K=64 each)
                    nc.tensor.matmul(
                        pss[:, 0:QC], lhsT=KT[0:64, kcol:kcol + 128],
                        rhs=QT[0:64, q0:q0 + QC], start=True, stop=True)
                    nc.tensor.matmul(
                        pss[:, QC:2 * QC], lhsT=KT[64:128, kcol:kcol + 128],
                        rhs=QT[64:128, q0:q0 + QC], start=True, stop=True)
                    eb = epool.tile([128, 2 * QC], BF16, tag="eb")
                    nc.scalar.activation(eb[:], pss[:], AF.Exp, scale=float(SCALE))
                    nc.tensor.matmul(psA[:], lhsT=vaug[b][0][:, k, :],
                                     rhs=eb[:, 0:QC],
                                     start=(k == 0), stop=(k == NKC - 1))
                    nc.tensor.matmul(psB[:], lhsT=vaug[b][1][:, k, :],
                                     rhs=eb[:, QC:2 * QC],
                                     start=(k == 0), stop=(k == NKC - 1))
                    if k in FIN_AT:
                        emit_finish_stage(prev, FIN_AT[k])
                    for fn in filler.get(k, ()):
                        fn()
                # drain each accumulator with ONE full-tile copy so psA/psB
                # release ASAP (the next iteration's attn@V needs the slots).
                # odA = [o^T_h0 (0:64) | denom_h0 (64:128)], odB the mirror.
                odA = work.tile([128, QC], F32R, tag="odA")
                odB = work.tile([128, QC], F32R, tag="odB")
                nc.vector.tensor_copy(odA[:], psA[:])
                nc.vector.tensor_copy(odB[:], psB[:])
                emit_finish_stage(prev, 4)
                emit_finish_stage(prev, 5)
                return {"q0": q0, "odA": odA, "odB": odB}

            # ---- emission order: qkv/vtrans for batch 1 are spread through
            # batch 0's attention k-loops as PE filler; each iteration's
            # finish-phase is pipelined into the next k-loop ----
            for s in range(4):
                emit_qkv(s)
            for j in range(NKC):
                emit_vtrans(j)
            prev = None
            for q in range(4):
                prev = emit_attn(0, q, prev)
                s = 4 + q
                emit_qkv(s)
                for j in range(4 * s, 4 * s + 4):
                    emit_vtrans(j)
            for q in range(4):
                prev = emit_attn(1, q, prev)
            for stage in range(4):
                emit_finish_stage(prev, stage)
            emit_finish_stage(prev, 4)
            emit_finish_stage(prev, 5)

    nc.compile()
    return nc


def _get_nc():
    if "nc" not in _cache:
        _cache["nc"] = _build()
    return _cache["nc"]


def _prep_inputs(x, w_qkv, w_out):
    import ml_dtypes
    bf16 = ml_dtypes.bfloat16
    x = np.asarray(x, dtype=np.float32)
    w_qkv = np.asarray(w_qkv, dtype=np.float32)
    w_out = np.asarray(w_out, dtype=np.float32)
    xT = np.ascontiguousarray(x.reshape(BS, D).T.astype(bf16))
    in_maps = []
    for c in range(N_CORES):
        # reference splits qkv as (v, q, k): v rows [0,D), q [D,2D), k [2D,3D)
        wq = w_qkv[D + 128 * c: D + 128 * (c + 1)]
        wk = w_qkv[2 * D + 128 * c: 2 * D + 128 * (c + 1)]
        wv = w_qkv[128 * c: 128 * (c + 1)]
        wqkvT = np.ascontiguousarray(
            np.concatenate([wq, wk, wv], axis=0).T.astype(bf16))
        woutT = np.ascontiguousarray(
            w_out[:, 128 * c:128 * (c + 1)].T.astype(bf16))
        in_maps.append({"xT": xT, "wqkvT": wqkvT, "woutT": woutT})
    return in_maps


def kernel(x, w_qkv, w_out, b_out):
    from concourse.bass_utils import run_bass_kernel_spmd

    nc = _get_nc()
    in_maps = _prep_inputs(x, w_qkv, w_out)
    b_out = np.asarray(b_out, dtype=np.float32)
    res = run_bass_kernel_spmd(nc, in_maps, core_ids=list(range(N_CORES)))
    acc = np.zeros((BS, D), np.float32)
    for c in range(N_CORES):
        acc += res.results[c]["out"].astype(np.float32)
    acc = acc + b_out[None, :]
    return acc.reshape(B, S, D)


# revision 27
# speedup vs baseline: 1.5199x; 1.0001x over previous
"""Trainium2 Bass kernel for nn_Attention_51548197487430.

Multi-head attention (B=2, S=2048, D=1024, H=16, HD=64), fp32 reference,
sharded 2 heads per core across 8 NeuronCores (head/tensor parallel per
the sharding hint: w_qkv output dim and w_out input dim split per-head;
the all-reduce after the output projection is realized as the host-side
unshard step, which sums the 8 partial outputs).

Per-core device kernel (SPMD, identical program; per-core weight slices):
  inputs (host pre-laid-out, cast to bf16):
    xT     (1024, 4096)  x^T, both batches side by side   [same on all cores]
    wqkvT  (1024, 384)   [Wq_h0|Wq_h1|Wk_h0|Wk_h1|Wv_h0|Wv_h1]^T for this core
    woutT  (128, 1024)   w_out[:, 128c:128c+128]^T
  output:
    out    (4096, 1024)  fp32 partial x_out (pre-bias); host sums over cores

  1. QKV^T projection -> QT/KT/VT (128, 4096); matmuls bf16 (fp32 PSUM).
  2. V^T -> V via PE transpose; build V_aug per head: h0=[V|ones],
     h1=[ones|V] - the ones-columns compute the softmax denominator in
     the same matmul as attn@V, on the partition half opposite to o^T.
  3. Per (batch, q-512-chunk), loop 16 k-chunks: scores^T by row-packed
     matmul pairs (two heads in PE row groups 0-1/2-3 concurrently), one
     fused exp over both heads' psum banks (scale folded in; max
     subtraction skipped - scores are O(+-6) so fp32 exp is safe),
     attn@V accumulation.  Normalization: PE broadcast of the denominator
     rows + DVE reciprocal + multiply, off the critical path.
  4. Output projection (contraction over this core's 128 o-dims).
"""

import numpy as np

B, S, D, H, HD = 2, 2048, 1024, 16, 64
N_CORES = 8
SCALE = HD ** (-0.5)
BS = B * S               # 4096
SC = 512                 # qkv-phase s-chunk (8 chunks)
QC = 512                 # attention q-chunk
NKC = S // 128           # 16 k-chunks per batch
DC = D // 128            # 8 contraction chunks

_cache = {}


def _build():
    import concourse.bass as bass
    import concourse.mybir as mybir
    import concourse.tile as tile
    from concourse import bacc

    F32 = mybir.dt.float32
    F32R = mybir.dt.float32r
    BF16 = mybir.dt.bfloat16
    F16 = mybir.dt.float16
    AF = mybir.ActivationFunctionType

    nc = bacc.Bacc("TRN2", target_bir_lowering=False, debug=False,
                   num_devices=N_CORES)
    xT_d = nc.dram_tensor("xT", (D, BS), BF16, kind="ExternalInput").ap()
    wqkvT_d = nc.dram_tensor("wqkvT", (D, 384), BF16, kind="ExternalInput").ap()
    woutT_d = nc.dram_tensor("woutT", (128, D), BF16, kind="ExternalInput").ap()
    # fp16 partials: |values| << 1, so fp16's 10-bit mantissa beats bf16
    # and halves the output DMA; host upcasts and sums in fp32.
    out_d = nc.dram_tensor("out", (BS, D), F16, kind="ExternalOutput").ap()

    with tile.TileContext(nc) as tc:
        with tc.tile_pool(name="persist", bufs=1) as persist, \
             tc.tile_pool(name="xin", bufs=2) as xin, \
             tc.tile_pool(name="epool", bufs=3) as epool, \
             tc.tile_pool(name="work", bufs=2) as work, \
             tc.tile_pool(name="ps_sc", bufs=2, space="PSUM") as ps_sc, \
             tc.tile_pool(name="pacc", bufs=4, space="PSUM") as pacc:

            # ---- persistent tiles ----
            wqkvT = persist.tile([128, DC, 384], BF16, tag="wqkvT")
            nc.sync.dma_start(wqkvT[:], wqkvT_d.rearrange("(po pi) e -> pi po e", pi=128))
            woutT = persist.tile([128, D], BF16, tag="woutT")
            nc.sync.dma_start(woutT[:], woutT_d)

            ident = persist.tile([128, 128], F32, tag="ident")
            from concourse.masks import make_identity
            make_identity(nc, ident[:])

            QT = persist.tile([128, BS], BF16, tag="QT")
            KT = persist.tile([128, BS], BF16, tag="KT")
            VT = persist.tile([128, BS], F32, tag="VT")
            # V_aug[b][h]: (128, NKC, 128); h0 = [V | ones], h1 = [ones | V]
            vaug = [[persist.tile([128, NKC, 128], BF16, tag=f"vaug{b}{h}",
                                  name=f"vaug{b}{h}")
                     for h in range(2)] for b in range(B)]
            # constants: memset supports fp32 only; DVE copy rounds/casts
            const_f32 = persist.tile([128, NKC * 64], F32, tag="const_f32")
            nc.gpsimd.memset(const_f32[:], 1.0)
            # inv2: anti-block-diagonal 1/64 weights; one matmul pair
            # accumulates both heads' denominator broadcasts into ONE psum
            # tile (h0's denom -> partitions 0-63, h1's -> 64-127).
            inv2 = persist.tile([128, 128], F32R, tag="inv2")
            inv2_f32 = persist.tile([128, 128], F32, tag="inv2_f32")
            nc.gpsimd.memset(inv2_f32[:], 0.0)
            nc.gpsimd.memset(inv2_f32[64:128, 0:64], 1.0 / 64.0)
            nc.gpsimd.memset(inv2_f32[0:64, 64:128], 1.0 / 64.0)
            nc.vector.tensor_copy(inv2[:], inv2_f32[:])
            ones_3d = const_f32[:].rearrange("p (a b) -> p a b", b=64)
            for b in range(B):
                nc.vector.tensor_copy(vaug[b][0][:, :, 64:128], ones_3d)
                nc.vector.tensor_copy(vaug[b][1][:, :, 0:64], ones_3d)

            xts = {}

            def emit_xt_dma(s):
                xt = xin.tile([128, DC, SC], BF16, tag="xt", name="xt")
                nc.sync.dma_start(
                    xt[:], xT_d[:, s * SC:(s + 1) * SC]
                    .rearrange("(po pi) s -> pi po s", pi=128))
                xts[s] = xt

            def emit_qkv_part(s, e):
                """One e-chunk (Q, K or V) of the projection for s-chunk s."""
                dst = (QT, KT, VT)[e]
                ps = pacc.tile([128, SC], F32, tag="pacc", name="qkv_ps")
                for d in range(DC):
                    nc.tensor.matmul(
                        ps[:], lhsT=wqkvT[:, d, 128 * e:128 * (e + 1)],
                        rhs=xts[s][:, d, :], start=(d == 0), stop=(d == DC - 1))
                nc.vector.tensor_copy(dst[:, s * SC:(s + 1) * SC], ps[:])

            def emit_qkv(s):
                """Project x^T cols [s*512, (s+1)*512) -> QT/KT/VT."""
                emit_xt_dma(s)
                for e in range(3):
                    emit_qkv_part(s, e)

            def emit_vtrans(j):
                """Transpose VT cols [j*128,(j+1)*128) into V_aug tiles."""
                b, k = divmod(j, NKC)
                ps = pacc.tile([128, SC], F32, tag="pacc")
                pt = ps[:, 0:128]
                nc.tensor.transpose(pt, VT[:, j * 128:(j + 1) * 128], ident[:])
                nc.vector.tensor_copy(vaug[b][0][:, k, 0:64], pt[:, 0:64])
                nc.vector.tensor_copy(vaug[b][1][:, k, 64:128], pt[:, 64:128])

            def emit_finish_stage(st, stage):
                """Software-pipelined tail of a previous attention iteration,
                interleaved into the current k-loop so the PE's static
                instruction order never blocks on the DVE normalize chain."""
                if st is None:
                    return
                if stage == 0:
                    # both heads' denominator broadcasts accumulate into ONE
                    # psum tile (anti-block-diagonal inv2) -> one reciprocal
                    st["invd"] = work.tile([128, QC], F32, tag="invd", name="invd")
                    st["pbc"] = pacc.tile([128, SC], F32, tag="pacc", name="pbc")
                    nc.tensor.matmul(st["pbc"][:], lhsT=inv2[64:128, :],
                                     rhs=st["odA"][64:128, :],
                                     start=True, stop=False)
                    nc.tensor.matmul(st["pbc"][:], lhsT=inv2[0:64, :],
                                     rhs=st["odB"][0:64, :],
                                     start=False, stop=True)
                    nc.vector.reciprocal(st["invd"][:], st["pbc"][:])
                elif stage == 1:
                    st["ot"] = work.tile([128, QC], BF16, tag="ot", name="ot")
                    nc.vector.tensor_mul(out=st["ot"][0:64, :],
                                         in0=st["odA"][0:64, :].bitcast(F32),
                                         in1=st["invd"][0:64, :])
                    nc.vector.tensor_mul(out=st["ot"][64:128, :],
                                         in0=st["odB"][64:128, :].bitcast(F32),
                                         in1=st["invd"][64:128, :])
                else:
                    # stages 2..5: output projection, one 128-row chunk each
                    j = stage - 2
                    osb = work.tile([128, D], F16, tag="osb")
                    for e in range(D // SC):
                        po = pacc.tile([128, SC], F32, tag="pacc")
                        nc.tensor.matmul(
                            po[:], lhsT=st["ot"][:, j * 128:(j + 1) * 128],
                            rhs=woutT[:, e * SC:(e + 1) * SC],
                            start=True, stop=True)
                        nc.vector.tensor_copy(osb[:, e * SC:(e + 1) * SC], po[:])
                    row = st["q0"] + j * 128
                    nc.sync.dma_start(out_d[row:row + 128, :], osb[:])

            # k-index -> pipelined finish stage of the previous iteration
            # (stages 4-5 = outproj j2/j3 run at the iteration boundary
            # as PE filler while the accumulators drain)
            FIN_AT = {4: 0, 6: 1, 8: 2, 10: 3}

            def emit_attn(b, q, prev, filler=None):
                """One (batch, q-512-chunk): softmax(QK^T*scale)V.  `filler`
                maps k -> list of closures (qkv/vtrans work spread through the
                k-loop to even out PE duty).  Returns state for the pipelined
                finish (normalize + out projection)."""
                filler = filler or {}
                q0 = b * S + q * QC          # column base in QT (global s)
                psA = pacc.tile([128, SC], F32, tag="pacc")  # h0: oT | denom
                psB = pacc.tile([128, SC], F32, tag="pacc")  # h1: denom | oT
                for k in range(NKC):
                    kcol = b * S + k * 128
                    pss = ps_sc.tile([128, 2 * QC], F32, tag="scores")
                    # scores^T, two heads packed in row groups (K=64 each)
                    nc.tensor.matmul(
                        pss[:, 0:QC], lhsT=KT[0:64, kcol:kcol + 128],
                        rhs=QT[0:64, q0:q0 + QC], start=True, stop=True)
                    nc.tensor.matmul(
                        pss[:, QC:2 * QC], lhsT=KT[64:128, kcol:kcol + 128],
                        rhs=QT[64:128, q0:q0 + QC], start=True, stop=True)
                    eb = epool.tile([128, 2 * QC], BF16, tag="eb")
                    nc.scalar.activation(eb[:], pss[:], AF.Exp, scale=float(SCALE))
                    nc.tensor.matmul(psA[:], lhsT=vaug[b][0][:, k, :],
                                     rhs=eb[:, 0:QC],
                                     start=(k == 0), stop=(k == NKC - 1))
                    nc.tensor.matmul(psB[:], lhsT=vaug[b][1][:, k, :],
                                     rhs=eb[:, QC:2 * QC],
                                     start=(k == 0), stop=(k == NKC - 1))
                    if k in FIN_AT:
                        emit_finish_stage(prev, FIN_AT[k])
                    for fn in filler.get(k, ()):
                        fn()
                # drain each accumulator with ONE full-tile copy so psA/psB
                # release ASAP (the next iteration's attn@V needs the slots).
                # odA = [o^T_h0 (0:64) | denom_h0 (64:128)], odB the mirror.
                odA = work.tile([128, QC], F32R, tag="odA")
                odB = work.tile([128, QC], F32R, tag="odB")
                nc.vector.tensor_copy(odA[:], psA[:])
                nc.vector.tensor_copy(odB[:], psB[:])
                emit_finish_stage(prev, 4)
                emit_finish_stage(prev, 5)
                return {"q0": q0, "odA": odA, "odB": odB}

            # ---- emission order: qkv/vtrans for batch 1 are spread through
            # batch 0's attention k-loops as PE filler; each iteration's
            # finish-phase is pipelined into the next k-loop ----
            for s in range(4):
                emit_qkv(s)
            for j in range(NKC):
                emit_vtrans(j)
            prev = None
            for q in range(4):
                prev = emit_attn(0, q, prev)
                s = 4 + q
                emit_qkv(s)
                for j in range(4 * s, 4 * s + 4):
                    emit_vtrans(j)
            for q in range(4):
                prev = emit_attn(1, q, prev)
            for stage in range(4):
                emit_finish_stage(prev, stage)
            emit_finish_stage(prev, 4)
            emit_finish_stage(prev, 5)

    nc.compile()
    return nc


def _get_nc():
    if "nc" not in _cache:
        _cache["nc"] = _build()
    return _cache["nc"]


def _prep_inputs(x, w_qkv, w_out):
    import ml_dtypes
    bf16 = ml_dtypes.bfloat16
    x = np.asarray(x, dtype=np.float32)
    w_qkv = np.asarray(w_qkv, dtype=np.float32)
    w_out = np.asarray(w_out, dtype=np.float32)
    xT = np.ascontiguousarray(x.reshape(BS, D).T.astype(bf16))
    in_maps = []
    for c in range(N_CORES):
        # reference splits qkv as (v, q, k): v rows [0,D), q [D,2D), k [2D,3D)
        wq = w_qkv[D + 128 * c: D + 128 * (c + 1)]
        wk = w_qkv[2 * D + 128 * c: 2 * D + 128 * (c + 1)]
        wv = w_qkv[128 * c: 128 * (c + 1)]
        wqkvT = np.ascontiguousarray(
            np.concatenate([wq, wk, wv], axis=0).T.astype(bf16))
        woutT = np.ascontiguousarray(
            w_out[:, 128 * c:128 * (c + 1)].T.astype(bf16))
        in_maps.append({"xT": xT, "wqkvT": wqkvT, "woutT": woutT})
    return in_maps


def kernel(x, w_qkv, w_out, b_out):
    from concourse.bass_utils import run_bass_kernel_spmd

    nc = _get_nc()
    in_maps = _prep_inputs(x, w_qkv, w_out)
    b_out = np.asarray(b_out, dtype=np.float32)
    res = run_bass_kernel_spmd(nc, in_maps, core_ids=list(range(N_CORES)))
    acc = np.zeros((BS, D), np.float32)
    for c in range(N_CORES):
        acc += res.results[c]["out"].astype(np.float32)
    acc = acc + b_out[None, :]
    return acc.reshape(B, S, D)
